# revision 4
# baseline (speedup 1.0000x reference)
"""Trainium2 Bass kernel for nn_DirectInjectionEncoder (moe_routing).

Strategy (8 NeuronCores):
  - The five projection GEMMs (Wk/Wv/Wgate/Wup/Wdown) are sharded over the
    output dim d_model=2560 -> 320 columns per core, so each core streams
    only 1/8 of the weights from HBM. Every core computes its 320-column
    slice of all 16*36=576 rows per group.
  - Row L2-norms need the full 2560-dim row, so each core computes partial
    sums of squares for its slice; one tiny 8-core AllGather (25 cols x 128
    partitions) distributes the partials and every core reconstructs the
    full norm locally before scaling its slice.
  - Identity tokens (9 of 14 slots/layer, first 2560 dims, no weights) are
    data-parallel over the batch: core c handles batches [2c, 2c+1] fully.
  - Host-side prep ("sharding") gathers token groups, pre-transposes the
    contraction dim onto partitions, and slices the weights per core.
"""

import os
import sys

sys.path.insert(0, "/opt/trn_rl_repo")

import numpy as np

from concourse import bacc, bass, mybir
from concourse.bass_utils import run_bass_kernel_spmd
from concourse.tile import TileContext

D_MODEL = 2560
NUM_LAYERS = 36
TOKENS_PER_LAYER = 14
EPS = 1e-8
B = 16
N_CORES = 8
CORE_IDS = list(range(N_CORES))
D_SHARD = D_MODEL // N_CORES  # 320
ROWS = B * NUM_LAYERS  # 576
ROW_TILES = [(0, 128), (128, 128), (256, 128), (384, 128), (512, 64)]

IDENTITY_OFFSETS = np.array([0, 1, 2, 4, 6, 7, 8, 10, 13])
# (offset, weight_name, in_dim) -- big groups first so PE work is dense early
PROJ_GROUPS = [
    (9, "Wgate", 10240),
    (11, "Wup", 10240),
    (12, "Wdown", 10240),
    (3, "Wk", 640),
    (5, "Wv", 640),
]
ID_ROWS = (B // N_CORES) * NUM_LAYERS * len(IDENTITY_OFFSETS)  # 648
ID_TILES = [(0, 128), (128, 128), (256, 128), (384, 128), (512, 128), (640, 8)]
N_SSQ_COLS = len(PROJ_GROUPS) * len(ROW_TILES)  # 25

F32 = mybir.dt.float32
AF = mybir.ActivationFunctionType


def _positions(offset):
    return np.arange(NUM_LAYERS) * TOKENS_PER_LAYER + offset


def build_program():
    nc = bacc.Bacc("TRN2", num_devices=N_CORES)

    xt_d, wt_d, om_d = [], [], []
    for gi, (off, wname, ind) in enumerate(PROJ_GROUPS):
        xt_d.append(nc.declare_dram_parameter(f"xt_{gi}", [ind, ROWS], F32, isOutput=False))
        wt_d.append(nc.declare_dram_parameter(f"wt_{gi}", [ind, D_SHARD], F32, isOutput=False))
        om_d.append(nc.declare_dram_parameter(f"om_{gi}", [ROWS, D_SHARD], F32, isOutput=True))
    idx_d = nc.declare_dram_parameter("id_x", [ID_ROWS, D_MODEL], F32, isOutput=False)
    ido_d = nc.declare_dram_parameter("out_id", [ID_ROWS, D_MODEL], F32, isOutput=True)

    with TileContext(nc) as tc:
        with (
            tc.tile_pool(name="xt", bufs=4) as xt_pool,
            tc.tile_pool(name="wt", bufs=4) as wt_pool,
            tc.tile_pool(name="sout", bufs=N_SSQ_COLS) as sout_pool,
            tc.tile_pool(name="scr", bufs=2) as scr_pool,
            tc.tile_pool(name="idp", bufs=3) as id_pool,
            tc.tile_pool(name="idscr", bufs=2) as idscr_pool,
            tc.tile_pool(name="small", bufs=1) as small_pool,
            tc.tile_pool(name="ps", bufs=8, space="PSUM") as psum_pool,
            tc.tile_pool(name="dram", bufs=1, space="DRAM") as dram_pool,
        ):
            ssq = small_pool.tile([128, N_SSQ_COLS], F32, tag="ssq")
            nc.vector.memset(ssq[:], 0.0)

            souts = {}
            for gi, (off, wname, ind) in enumerate(PROJ_GROUPS):
                nk = ind // 128
                ps = [psum_pool.tile([128, D_SHARD], F32, tag="ps", name=f"ps_{gi}_{ri}") for ri in range(len(ROW_TILES))]
                for kt in range(nk):
                    xt = xt_pool.tile([128, ROWS], F32, tag="xt")
                    wt = wt_pool.tile([128, D_SHARD], F32, tag="wt")
                    nc.sync.dma_start(out=xt[:], in_=xt_d[gi][kt * 128 : (kt + 1) * 128, :])
                    nc.sync.dma_start(out=wt[:], in_=wt_d[gi][kt * 128 : (kt + 1) * 128, :])
                    for r, (r0, rw) in enumerate(ROW_TILES):
                        nc.tensor.matmul(
                            ps[r][:rw, :],
                            xt[:, r0 : r0 + rw],
                            wt[:],
                            start=(kt == 0),
                            stop=(kt == nk - 1),
                        )
                for r, (r0, rw) in enumerate(ROW_TILES):
                    col = gi * len(ROW_TILES) + r
                    so = sout_pool.tile([128, D_SHARD], F32, tag="sout")
                    scr = scr_pool.tile([128, D_SHARD], F32, tag="scr")
                    nc.vector.tensor_copy(so[:rw, :], ps[r][:rw, :])
                    nc.scalar.activation(
                        scr[:rw, :], ps[r][:rw, :], AF.Square,
                        accum_out=ssq[:rw, col : col + 1],
                    )
                    souts[(gi, r)] = so

            # ---- cross-core exchange of partial sums of squares ----
            cc_in = dram_pool.tile([128, N_SSQ_COLS], F32, tag="ccin")
            cc_out = dram_pool.tile([N_CORES, 128, N_SSQ_COLS], F32, tag="ccout")
            nc.gpsimd.dma_start(out=cc_in[:], in_=ssq[:])
            nc.gpsimd.collective_compute(
                "AllGather",
                mybir.AluOpType.bypass,
                ins=[cc_in.opt()],
                outs=[cc_out.opt()],
                replica_groups=[CORE_IDS],
            )
            ag = small_pool.tile([128, N_CORES * N_SSQ_COLS], F32, tag="ag")
            for rr in range(N_CORES):
                nc.sync.dma_start(
                    out=ag[:, rr * N_SSQ_COLS : (rr + 1) * N_SSQ_COLS],
                    in_=cc_out[rr, :, :],
                )
            tsq = small_pool.tile([128, N_SSQ_COLS], F32, tag="tsq")
            nc.vector.tensor_add(tsq[:], ag[:, :N_SSQ_COLS], ag[:, N_SSQ_COLS : 2 * N_SSQ_COLS])
            for rr in range(2, N_CORES):
                nc.vector.tensor_add(
                    tsq[:], tsq[:], ag[:, rr * N_SSQ_COLS : (rr + 1) * N_SSQ_COLS]
                )
            norm = small_pool.tile([128, N_SSQ_COLS], F32, tag="norm")
            nc.scalar.sqrt(norm[:], tsq[:])
            nc.scalar.activation(norm[:], norm[:], AF.Copy, bias=EPS)
            scale = small_pool.tile([128, N_SSQ_COLS], F32, tag="scale")
            nc.vector.reciprocal(scale[:], norm[:])

            for gi in range(len(PROJ_GROUPS)):
                for r, (r0, rw) in enumerate(ROW_TILES):
                    col = gi * len(ROW_TILES) + r
                    so = souts[(gi, r)]
                    nc.scalar.activation(
                        so[:rw, :], so[:rw, :], AF.Copy,
                        scale=scale[:rw, col : col + 1],
                    )
                    nc.sync.dma_start(out=om_d[gi][r0 : r0 + rw, :], in_=so[:rw, :])

            # ---- identity tokens (batch-parallel; norms are core-local) ----
            # Each tile's norm chain is independent so id_pool slots recycle.
            for t, (t0, tw) in enumerate(ID_TILES):
                it = id_pool.tile([128, D_MODEL], F32, tag="idp", name=f"idp_{t}")
                iscr = idscr_pool.tile([128, D_MODEL], F32, tag="idscr", name=f"idscr_{t}")
                nc.sync.dma_start(out=it[:tw, :], in_=idx_d[t0 : t0 + tw, :])
                issq = small_pool.tile([128, 1], F32, tag=f"idssq{t}", name=f"idssq_{t}")
                nc.scalar.activation(
                    iscr[:tw, :], it[:tw, :], AF.Square,
                    accum_out=issq[:tw, :],
                )
                inorm = small_pool.tile([128, 1], F32, tag=f"idnorm{t}", name=f"idnorm_{t}")
                nc.scalar.sqrt(inorm[:tw, :], issq[:tw, :])
                nc.scalar.activation(inorm[:tw, :], inorm[:tw, :], AF.Copy, bias=EPS)
                iscale = small_pool.tile([128, 1], F32, tag=f"idscale{t}", name=f"idscale_{t}")
                nc.vector.reciprocal(iscale[:tw, :], inorm[:tw, :])
                nc.scalar.activation(
                    it[:tw, :], it[:tw, :], AF.Copy,
                    scale=iscale[:tw, :],
                )
                nc.sync.dma_start(out=ido_d[t0 : t0 + tw, :], in_=it[:tw, :])

    nc.compile()
    return nc


_NC = None


def _get_nc():
    global _NC
    if _NC is None:
        _NC = build_program()
    return _NC


def _prep_inputs(lora_tokens, weights):
    """Host-side sharding: gather token groups, transpose contraction onto
    partitions, slice weights per core."""
    lora = np.ascontiguousarray(lora_tokens)
    shared = {}
    for gi, (off, wname, ind) in enumerate(PROJ_GROUPS):
        pos = _positions(off)
        x = lora[:, pos, :ind].reshape(ROWS, ind)
        shared[f"xt_{gi}"] = np.ascontiguousarray(x.T)

    id_pos = np.sort(np.concatenate([_positions(o) for o in IDENTITY_OFFSETS]))
    in_maps = []
    bpc = B // N_CORES
    for c in range(N_CORES):
        m = dict(shared)
        for gi, (off, wname, ind) in enumerate(PROJ_GROUPS):
            wsl = weights[wname][c * D_SHARD : (c + 1) * D_SHARD, :]  # [320, ind]
            m[f"wt_{gi}"] = np.ascontiguousarray(wsl.T)  # [ind, 320]
        m["id_x"] = np.ascontiguousarray(
            lora[c * bpc : (c + 1) * bpc, :, :][:, id_pos, :D_MODEL]
        ).reshape(ID_ROWS, D_MODEL)
        in_maps.append(m)
    return in_maps, id_pos


def run(inputs, trace=False):
    nc = _get_nc()
    weights = {k: inputs[k] for k in ("Wk", "Wv", "Wgate", "Wup", "Wdown")}
    in_maps, id_pos = _prep_inputs(inputs["lora_tokens"], weights)
    res = run_bass_kernel_spmd(nc, in_maps, CORE_IDS, trace=trace)

    out = np.zeros((B, NUM_LAYERS * TOKENS_PER_LAYER, D_MODEL), dtype=np.float32)
    bpc = B // N_CORES
    for c in range(N_CORES):
        r = res.results[c]
        out[c * bpc : (c + 1) * bpc, id_pos, :] = r["out_id"].reshape(
            bpc, len(id_pos), D_MODEL
        )
        for gi, (off, wname, ind) in enumerate(PROJ_GROUPS):
            pos = _positions(off)
            out[:, pos, c * D_SHARD : (c + 1) * D_SHARD] = r[f"om_{gi}"].reshape(
                B, NUM_LAYERS, D_SHARD
            )
    return out, res


def kernel(**inputs) -> np.ndarray:
    out, _ = run(inputs, trace=False)
    return out


# revision 5
# speedup vs baseline: 2.8200x; 2.8200x over previous
"""Trainium2 Bass kernel for nn_DirectInjectionEncoder (moe_routing).

Strategy (8 NeuronCores):
  - The five projection GEMMs (Wk/Wv/Wgate/Wup/Wdown) are sharded over the
    output dim d_model=2560 -> 320 columns per core, so each core streams
    only 1/8 of the weights from HBM. Every core computes its 320-column
    slice of all 16*36=576 rows per group.
  - Row L2-norms need the full 2560-dim row, so each core computes partial
    sums of squares for its slice; one tiny 8-core AllGather (25 cols x 128
    partitions) distributes the partials and every core reconstructs the
    full norm locally before scaling its slice.
  - Identity tokens (9 of 14 slots/layer, first 2560 dims, no weights) are
    data-parallel over the batch: core c handles batches [2c, 2c+1] fully.
  - Host-side prep ("sharding") gathers token groups, pre-transposes the
    contraction dim onto partitions, and slices the weights per core.
  - GEMM operands stream as bf16 (KERNEL_GEMM_DT=f32r/f32 to override);
    PSUM accumulation and the norm math stay fp32. fp32 matmul runs at 1/4
    PE rate on TRN2, so bf16/f32r is 4x PE throughput; bf16 also halves
    the dominant DMA traffic.
"""

import os
import sys

sys.path.insert(0, "/opt/trn_rl_repo")

import numpy as np
import ml_dtypes

from concourse import bacc, bass, mybir
from concourse.bass_utils import run_bass_kernel_spmd
from concourse.tile import TileContext

D_MODEL = 2560
NUM_LAYERS = 36
TOKENS_PER_LAYER = 14
EPS = 1e-8
B = 16
N_CORES = 8
CORE_IDS = list(range(N_CORES))
D_SHARD = D_MODEL // N_CORES  # 320
ROWS = B * NUM_LAYERS  # 576
ROW_TILES = [(0, 128), (128, 128), (256, 128), (384, 128), (512, 64)]

IDENTITY_OFFSETS = np.array([0, 1, 2, 4, 6, 7, 8, 10, 13])
# (offset, weight_name, in_dim) -- big groups first so PE work is dense early
PROJ_GROUPS = [
    (9, "Wgate", 10240),
    (11, "Wup", 10240),
    (12, "Wdown", 10240),
    (3, "Wk", 640),
    (5, "Wv", 640),
]
ID_ROWS = (B // N_CORES) * NUM_LAYERS * len(IDENTITY_OFFSETS)  # 648
ID_TILES = [(0, 128), (128, 128), (256, 128), (384, 128), (512, 128), (640, 8)]
N_SSQ_COLS = len(PROJ_GROUPS) * len(ROW_TILES)  # 25

F32 = mybir.dt.float32
AF = mybir.ActivationFunctionType

GEMM_MODE = os.environ.get("KERNEL_GEMM_DT", "bf16")
if GEMM_MODE == "bf16":
    GEMM_DT = mybir.dt.bfloat16
    GEMM_NP = ml_dtypes.bfloat16
    KB_BIG = 4  # k-tiles per DMA super-tile (~590 KB per xt transfer)
elif GEMM_MODE == "f32r":
    GEMM_DT = mybir.dt.float32r
    GEMM_NP = np.float32
    KB_BIG = 2
else:
    GEMM_DT = mybir.dt.float32
    GEMM_NP = np.float32
    KB_BIG = 2


def _positions(offset):
    return np.arange(NUM_LAYERS) * TOKENS_PER_LAYER + offset


def _kbatch(ind):
    # k-tiles per super-tile for a group with contraction dim `ind`
    return 5 if ind == 640 else KB_BIG


def build_program():
    nc = bacc.Bacc("TRN2", num_devices=N_CORES)

    xt_d, wt_d, om_d = [], [], []
    for gi, (off, wname, ind) in enumerate(PROJ_GROUPS):
        xt_d.append(nc.declare_dram_parameter(f"xt_{gi}", [ind, ROWS], GEMM_DT, isOutput=False))
        wt_d.append(nc.declare_dram_parameter(f"wt_{gi}", [ind, D_SHARD], GEMM_DT, isOutput=False))
        om_d.append(nc.declare_dram_parameter(f"om_{gi}", [ROWS, D_SHARD], F32, isOutput=True))
    idx_d = nc.declare_dram_parameter("id_x", [ID_ROWS, D_MODEL], F32, isOutput=False)
    ido_d = nc.declare_dram_parameter("out_id", [ID_ROWS, D_MODEL], F32, isOutput=True)

    with TileContext(nc) as tc:
        with (
            tc.tile_pool(name="xt", bufs=3) as xt_pool,
            tc.tile_pool(name="wt", bufs=3) as wt_pool,
            tc.tile_pool(name="sout", bufs=N_SSQ_COLS) as sout_pool,
            tc.tile_pool(name="scr", bufs=2) as scr_pool,
            tc.tile_pool(name="idp", bufs=3) as id_pool,
            tc.tile_pool(name="idscr", bufs=2) as idscr_pool,
            tc.tile_pool(name="small", bufs=1) as small_pool,
            tc.tile_pool(name="ps", bufs=8, space="PSUM") as psum_pool,
            tc.tile_pool(name="dram", bufs=1, space="DRAM") as dram_pool,
        ):
            ssq = small_pool.tile([128, N_SSQ_COLS], F32, tag="ssq")
            nc.vector.memset(ssq[:], 0.0)

            souts = {}
            for gi, (off, wname, ind) in enumerate(PROJ_GROUPS):
                nk = ind // 128
                kb = _kbatch(ind)
                nsup = nk // kb
                ps = [
                    psum_pool.tile([128, D_SHARD], F32, tag="ps", name=f"ps_{gi}_{ri}")
                    for ri in range(len(ROW_TILES))
                ]
                xt_view = xt_d[gi].rearrange("(j kb p) c -> j p kb c", kb=kb, p=128)
                wt_view = wt_d[gi].rearrange("(j kb p) c -> j p kb c", kb=kb, p=128)
                for j in range(nsup):
                    xt = xt_pool.tile([128, kb, ROWS], GEMM_DT, tag="xt", name=f"xt_{gi}_{j}")
                    wt = wt_pool.tile([128, kb, D_SHARD], GEMM_DT, tag="wt", name=f"wt_{gi}_{j}")
                    nc.sync.dma_start(out=xt[:], in_=xt_view[j])
                    nc.sync.dma_start(out=wt[:], in_=wt_view[j])
                    for k in range(kb):
                        kt = j * kb + k
                        for r, (r0, rw) in enumerate(ROW_TILES):
                            nc.tensor.matmul(
                                ps[r][:rw, :],
                                xt[:, k, r0 : r0 + rw],
                                wt[:, k, :],
                                start=(kt == 0),
                                stop=(kt == nk - 1),
                            )
                for r, (r0, rw) in enumerate(ROW_TILES):
                    col = gi * len(ROW_TILES) + r
                    so = sout_pool.tile([128, D_SHARD], F32, tag="sout", name=f"so_{gi}_{r}")
                    scr = scr_pool.tile([128, D_SHARD], F32, tag="scr", name=f"scr_{gi}_{r}")
                    nc.vector.tensor_copy(so[:rw, :], ps[r][:rw, :])
                    nc.scalar.activation(
                        scr[:rw, :], ps[r][:rw, :], AF.Square,
                        accum_out=ssq[:rw, col : col + 1],
                    )
                    souts[(gi, r)] = so

            # ---- cross-core exchange of partial sums of squares ----
            cc_in = dram_pool.tile([128, N_SSQ_COLS], F32, tag="ccin")
            cc_out = dram_pool.tile([N_CORES, 128, N_SSQ_COLS], F32, tag="ccout")
            nc.gpsimd.dma_start(out=cc_in[:], in_=ssq[:])
            nc.gpsimd.collective_compute(
                "AllGather",
                mybir.AluOpType.bypass,
                ins=[cc_in.opt()],
                outs=[cc_out.opt()],
                replica_groups=[CORE_IDS],
            )
            ag = small_pool.tile([128, N_CORES * N_SSQ_COLS], F32, tag="ag")
            for rr in range(N_CORES):
                nc.sync.dma_start(
                    out=ag[:, rr * N_SSQ_COLS : (rr + 1) * N_SSQ_COLS],
                    in_=cc_out[rr, :, :],
                )
            tsq = small_pool.tile([128, N_SSQ_COLS], F32, tag="tsq")
            nc.vector.tensor_add(tsq[:], ag[:, :N_SSQ_COLS], ag[:, N_SSQ_COLS : 2 * N_SSQ_COLS])
            for rr in range(2, N_CORES):
                nc.vector.tensor_add(
                    tsq[:], tsq[:], ag[:, rr * N_SSQ_COLS : (rr + 1) * N_SSQ_COLS]
                )
            norm = small_pool.tile([128, N_SSQ_COLS], F32, tag="norm")
            nc.scalar.sqrt(norm[:], tsq[:])
            nc.scalar.activation(norm[:], norm[:], AF.Copy, bias=EPS)
            scale = small_pool.tile([128, N_SSQ_COLS], F32, tag="scale")
            nc.vector.reciprocal(scale[:], norm[:])

            for gi in range(len(PROJ_GROUPS)):
                for r, (r0, rw) in enumerate(ROW_TILES):
                    col = gi * len(ROW_TILES) + r
                    so = souts[(gi, r)]
                    nc.scalar.activation(
                        so[:rw, :], so[:rw, :], AF.Copy,
                        scale=scale[:rw, col : col + 1],
                    )
                    nc.sync.dma_start(out=om_d[gi][r0 : r0 + rw, :], in_=so[:rw, :])

            # ---- identity tokens (batch-parallel; norms are core-local) ----
            # Each tile's norm chain is independent so id_pool slots recycle.
            for t, (t0, tw) in enumerate(ID_TILES):
                it = id_pool.tile([128, D_MODEL], F32, tag="idp", name=f"idp_{t}")
                iscr = idscr_pool.tile([128, D_MODEL], F32, tag="idscr", name=f"idscr_{t}")
                nc.sync.dma_start(out=it[:tw, :], in_=idx_d[t0 : t0 + tw, :])
                issq = small_pool.tile([128, 1], F32, tag=f"idssq{t}", name=f"idssq_{t}")
                nc.scalar.activation(
                    iscr[:tw, :], it[:tw, :], AF.Square,
                    accum_out=issq[:tw, :],
                )
                inorm = small_pool.tile([128, 1], F32, tag=f"idnorm{t}", name=f"idnorm_{t}")
                nc.scalar.sqrt(inorm[:tw, :], issq[:tw, :])
                nc.scalar.activation(inorm[:tw, :], inorm[:tw, :], AF.Copy, bias=EPS)
                iscale = small_pool.tile([128, 1], F32, tag=f"idscale{t}", name=f"idscale_{t}")
                nc.vector.reciprocal(iscale[:tw, :], inorm[:tw, :])
                nc.scalar.activation(
                    it[:tw, :], it[:tw, :], AF.Copy,
                    scale=iscale[:tw, :],
                )
                nc.sync.dma_start(out=ido_d[t0 : t0 + tw, :], in_=it[:tw, :])

    nc.compile()
    return nc


_NC = None


def _get_nc():
    global _NC
    if _NC is None:
        _NC = build_program()
    return _NC


def _prep_inputs(lora_tokens, weights):
    """Host-side sharding: gather token groups, transpose contraction onto
    partitions, slice weights per core."""
    lora = np.ascontiguousarray(lora_tokens)
    shared = {}
    for gi, (off, wname, ind) in enumerate(PROJ_GROUPS):
        pos = _positions(off)
        x = lora[:, pos, :ind].reshape(ROWS, ind)
        shared[f"xt_{gi}"] = np.ascontiguousarray(x.T.astype(GEMM_NP))

    id_pos = np.sort(np.concatenate([_positions(o) for o in IDENTITY_OFFSETS]))
    in_maps = []
    bpc = B // N_CORES
    for c in range(N_CORES):
        m = dict(shared)
        for gi, (off, wname, ind) in enumerate(PROJ_GROUPS):
            wsl = weights[wname][c * D_SHARD : (c + 1) * D_SHARD, :]  # [320, ind]
            m[f"wt_{gi}"] = np.ascontiguousarray(wsl.T.astype(GEMM_NP))  # [ind, 320]
        m["id_x"] = np.ascontiguousarray(
            lora[c * bpc : (c + 1) * bpc, :, :][:, id_pos, :D_MODEL]
        ).reshape(ID_ROWS, D_MODEL)
        in_maps.append(m)
    return in_maps, id_pos


def run(inputs, trace=False):
    nc = _get_nc()
    weights = {k: inputs[k] for k in ("Wk", "Wv", "Wgate", "Wup", "Wdown")}
    in_maps, id_pos = _prep_inputs(inputs["lora_tokens"], weights)
    res = run_bass_kernel_spmd(nc, in_maps, CORE_IDS, trace=trace)

    out = np.zeros((B, NUM_LAYERS * TOKENS_PER_LAYER, D_MODEL), dtype=np.float32)
    bpc = B // N_CORES
    for c in range(N_CORES):
        r = res.results[c]
        out[c * bpc : (c + 1) * bpc, id_pos, :] = r["out_id"].reshape(
            bpc, len(id_pos), D_MODEL
        )
        for gi, (off, wname, ind) in enumerate(PROJ_GROUPS):
            pos = _positions(off)
            out[:, pos, c * D_SHARD : (c + 1) * D_SHARD] = r[f"om_{gi}"].reshape(
                B, NUM_LAYERS, D_SHARD
            )
    return out, res


def kernel(**inputs) -> np.ndarray:
    out, _ = run(inputs, trace=False)
    return out
